# revision 11
# baseline (speedup 1.0000x reference)
"""MoE SwiGLU expert kernel for Trainium2, 8 NeuronCores.

Problem: x[4,2048,4096] routed through K=4 active experts (of 16):
    g = x @ gate[k], u = x @ up[k], act = silu(g)*u, out = act @ down[k]
    out[b,s,k,h], float32 in/out.

Sharding (8 cores): 8-way over tokens, full E per core.
  core c -> tokens [1024*c, 1024*c+1024), all 4 experts, E=1792.
No partial sums, each core owns its out[tok_slice, :, :] slab.

Matmuls run in bf16 (inputs rounded on host) with fp32 PSUM
accumulation and fp32 output. bf16 keeps the PE at 1 cycle/row like
float32r but re-enables the compiler's fast-weight-load path (FWL is
disabled for fp32-family stationary operands), which removes the
~14 ns/matmul LDWEIGHTS tail the fp32r version paid. End-to-end
rel-err vs the fp32 reference is ~6e-3 (gate: 2e-2).

Device-side layout per core (host pre-packs so every DMA line is a
contiguous >=2KB run per partition):
  xp  [8, 128, 4096]   bf16  x^T chunks: [xc, p, i*1024+t] with
                             h = xc*512 + i*128 + p, t in [0,1024)
  gwp [112, 128, 2048] bf16  gate chunks: [(k*14+j)*2+m, p, hh*128+e]
                             h = m*2048 + hh*128 + p, e_abs = j*128+e
  uwp [112, 128, 2048] bf16  up, same layout
  dwp [128, 128, 1792] bf16  down chunks: [k*32+i, p, j*128+h]
                             e_abs = j*128 + p, h_abs = i*128 + h
  out [128, 128, 1024] f32   [k*32+i, p, t]: out[t, k, i*128+p]

Compute loop per expert k: g/u phase (14 e-tiles j, PSUM [128,512]x2
per tensor, contraction over 32 h-tiles), silu*mul into resident bf16
act tiles, then down phase (32 h-tiles i, accumulating over 14 j).
Weight/x/down chunks stream through double-buffered pools; the next
expert's first g/u chunks prefetch on the scalar ring during the down
phase so the PE never waits at phase boundaries.
"""
import functools
import sys

sys.path.insert(0, "/opt/trn_rl_repo")

import numpy as np
import ml_dtypes

import concourse.bass as bass
import concourse.mybir as mybir
import concourse.tile as tile
from concourse import bacc
from concourse.bass_utils import run_bass_kernel_spmd

F32 = mybir.dt.float32
BF16 = mybir.dt.bfloat16
BF16NP = ml_dtypes.bfloat16

B, S, H, E, NEXP, K = 4, 2048, 4096, 1792, 16, 4
N_CORES = 8
TOK = B * S                  # 8192 tokens
TOK_PC = TOK // N_CORES      # 1024 tokens per core
N_ET = E // 128              # 14 e-tiles
N_HT = H // 128              # 32 h-tiles
TSUB = 512                   # PSUM moving free dim
N_TSUB = TOK_PC // TSUB      # 2
XCH = 4                      # h-tiles per x chunk
N_XCH = N_HT // XCH          # 8 x chunks
HH_PER_M = N_HT // 2         # 16 h-tiles per weight chunk


def _build(n_experts=K, n_etiles=N_ET, n_htiles=N_HT):
    """Build the per-core SPMD program."""
    nc = bacc.Bacc(
        "TRN2",
        target_bir_lowering=False,
        debug=False,
        enable_asserts=False,
        num_devices=N_CORES,
    )
    n_xch = n_htiles // XCH
    hh_per_m = n_htiles // 2
    xp = nc.dram_tensor("xp", [n_xch, 128, XCH * TOK_PC], BF16, kind="ExternalInput")
    gwp = nc.dram_tensor(
        "gwp", [n_experts * n_etiles * 2, 128, hh_per_m * 128], BF16,
        kind="ExternalInput",
    )
    uwp = nc.dram_tensor(
        "uwp", [n_experts * n_etiles * 2, 128, hh_per_m * 128], BF16,
        kind="ExternalInput",
    )
    dwp = nc.dram_tensor(
        "dwp", [n_experts * n_htiles, 128, n_etiles * 128], BF16,
        kind="ExternalInput",
    )
    out = nc.dram_tensor(
        "out", [n_experts * n_htiles, 128, TOK_PC], F32, kind="ExternalOutput"
    )

    silu = mybir.ActivationFunctionType.Silu

    with tile.TileContext(nc) as tc:
        with (
            tc.tile_pool(name="xpool", bufs=n_xch - 1) as xpool,
            tc.tile_pool(name="x0pool", bufs=XCH) as x0pool,
            tc.tile_pool(name="gupool", bufs=6) as gupool,
            tc.tile_pool(name="dpool", bufs=4) as dpool,
            tc.tile_pool(name="actpool", bufs=n_etiles) as actpool,
            tc.tile_pool(name="silpool", bufs=2) as silpool,
            tc.tile_pool(name="opool", bufs=4) as opool,
            tc.tile_pool(name="gups", bufs=6, space="PSUM") as gups,
            tc.tile_pool(name="ops", bufs=2, space="PSUM") as ops,
        ):
            def emit_x(xc, eng):
                xcht = xpool.tile([128, XCH * TOK_PC], BF16, tag="x", name="xcht")
                eng.dma_start(out=xcht, in_=xp[xc])
                return xcht

            def emit_wch(wdram, k, j, m, eng):
                wch = gupool.tile([128, hh_per_m * 128], BF16, tag="gu", name="wch")
                eng.dma_start(out=wch, in_=wdram[(k * n_etiles + j) * 2 + m])
                return wch

            # Cold start. The whole first e-tile is DMA-bound (x 8.4MB +
            # first weight chunks must land before it can finish), so spread
            # the loads over four rings and put the first weight chunk + the
            # h-tile-granular pieces of x chunk 0 in front.
            rings = [nc.gpsimd, nc.sync, nc.scalar]
            pre = {
                "g": emit_wch(gwp, 0, 0, 0, nc.sync),
                "u": emit_wch(uwp, 0, 0, 0, nc.scalar),
            }
            xchunks = [None] * n_xch
            xc0_parts = []
            for i in range(XCH):
                x0p = x0pool.tile([128, TOK_PC], BF16, tag="x0", name="x0p")
                rings[i % 3].dma_start(
                    out=x0p, in_=xp[0][:, i * TOK_PC : (i + 1) * TOK_PC]
                )
                xc0_parts.append(x0p)
            for xc in range(1, n_xch):
                xchunks[xc] = emit_x(xc, rings[(XCH + xc - 1) % 3])

            def xts_at(hi):
                # moving x slice for h-tile hi: [128, TOK_PC]
                xc, i = divmod(hi, XCH)
                if xc == 0:
                    return xc0_parts[i]
                return xchunks[xc][:, i * TOK_PC : (i + 1) * TOK_PC]

            for k in range(n_experts):
                def emit_dch(i, k=k):
                    dch = dpool.tile([128, n_etiles * 128], BF16, tag="d", name="dch")
                    nc.sync.dma_start(out=dch, in_=dwp[k * n_htiles + i])
                    return dch

                dch_pre = []
                act_tiles = []
                for j in range(n_etiles):
                    if j == n_etiles - 1:
                        # warm the down phase while the last e-tile's g/u
                        # matmuls still run
                        dch_pre = [emit_dch(i) for i in range(2)]
                    psg = [
                        gups.tile([128, TSUB], F32, tag="gups", name=f"psg{s}")
                        for s in range(N_TSUB)
                    ]
                    psu = [
                        gups.tile([128, TSUB], F32, tag="gups", name=f"psu{s}")
                        for s in range(N_TSUB)
                    ]
                    for m in range(2):
                        for wdram, ps in ((gwp, psg), (uwp, psu)):
                            if j == 0 and m == 0 and pre is not None:
                                wch = pre["g" if wdram is gwp else "u"]
                                if wdram is uwp:
                                    pre = None
                            else:
                                wch = emit_wch(
                                    wdram, k, j, m,
                                    nc.sync if wdram is gwp else nc.scalar,
                                )
                            # s innermost: consecutive matmuls share the
                            # stationary tile, so LDWEIGHTS runs once per
                            # h-tile instead of once per matmul
                            for hh in range(hh_per_m):
                                hi = m * hh_per_m + hh
                                for s in range(N_TSUB):
                                    nc.tensor.matmul(
                                        ps[s],
                                        wch[:, hh * 128 : (hh + 1) * 128],
                                        xts_at(hi)[:, s * TSUB : (s + 1) * TSUB],
                                        start=(hi == 0),
                                        stop=(hi == n_htiles - 1),
                                    )
                    act_j = actpool.tile([128, TOK_PC], BF16, tag="act", name="act_j")
                    for s in range(N_TSUB):
                        sil = silpool.tile([128, TSUB], F32, tag="sil", name="sil")
                        nc.scalar.activation(sil, psg[s], silu)
                        nc.vector.tensor_mul(
                            act_j[:, s * TSUB : (s + 1) * TSUB], sil, psu[s]
                        )
                    act_tiles.append(act_j)
                # next expert's first g/u chunks load on the scalar ring,
                # which sits idle through the down phase (sync carries dch)
                if k + 1 < n_experts:
                    pre = {
                        "g": emit_wch(gwp, k + 1, 0, 0, nc.scalar),
                        "u": emit_wch(uwp, k + 1, 0, 0, nc.scalar),
                    }
                for i in range(n_htiles):
                    dch = dch_pre[i] if i < len(dch_pre) else emit_dch(i)
                    pso = [
                        ops.tile([128, TSUB], F32, tag="ops", name=f"pso{s}")
                        for s in range(N_TSUB)
                    ]
                    # s outermost here: with only 2 PSUM banks, the s0 bank
                    # must finish early so its copy overlaps s1's matmuls
                    for s in range(N_TSUB):
                        for j in range(n_etiles):
                            nc.tensor.matmul(
                                pso[s],
                                dch[:, j * 128 : (j + 1) * 128],
                                act_tiles[j][:, s * TSUB : (s + 1) * TSUB],
                                start=(j == 0),
                                stop=(j == n_etiles - 1),
                            )
                    ot = opool.tile([128, TOK_PC], F32, tag="ot", name="ot")
                    if k == n_experts - 1 and i >= n_htiles - 2:
                        # kernel tail: ship each half as soon as it's copied
                        # so the final DMA isn't serialized behind both
                        for s in range(N_TSUB):
                            nc.vector.tensor_copy(
                                ot[:, s * TSUB : (s + 1) * TSUB], pso[s]
                            )
                            nc.gpsimd.dma_start(
                                out=out[k * n_htiles + i][
                                    :, s * TSUB : (s + 1) * TSUB
                                ],
                                in_=ot[:, s * TSUB : (s + 1) * TSUB],
                            )
                    else:
                        for s in range(N_TSUB):
                            nc.vector.tensor_copy(
                                ot[:, s * TSUB : (s + 1) * TSUB], pso[s]
                            )
                        nc.gpsimd.dma_start(out=out[k * n_htiles + i], in_=ot)
    nc.compile()
    return nc


@functools.cache
def _built_full():
    return _build()


def _pack_gu(w):
    # [K, H, E] f32 -> [(k*14+j)*2+m, p, hh*128+e] bf16 with
    # h = m*2048 + hh*128 + p, e_abs = j*128 + e
    w = w.reshape(K, 2, HH_PER_M, 128, N_ET, 128)        # k, m, hh, p, j, e
    w = w.transpose(0, 4, 1, 3, 2, 5)                    # k, j, m, p, hh, e
    return np.ascontiguousarray(
        w.reshape(K * N_ET * 2, 128, HH_PER_M * 128).astype(BF16NP)
    )


def _pack_d(w):
    # [K, E, H] f32 -> [k*32+i, p, j*128+h] bf16 with
    # e_abs = j*128 + p, h_abs = i*128 + h
    w = w.reshape(K, N_ET, 128, N_HT, 128)               # k, j, p, i, h
    w = w.transpose(0, 3, 2, 1, 4)                       # k, i, p, j, h
    return np.ascontiguousarray(
        w.reshape(K * N_HT, 128, N_ET * 128).astype(BF16NP)
    )


def kernel(x, gate_proj, up_proj, down_proj, expert_idx):
    x = np.asarray(x)
    idx = np.asarray(expert_idx)
    gate = np.asarray(gate_proj)[idx]  # [K, H, E]
    up = np.asarray(up_proj)[idx]
    down = np.asarray(down_proj)[idx]  # [K, E, H]

    nc = _built_full()

    gwp = _pack_gu(gate)
    uwp = _pack_gu(up)
    dwp = _pack_d(down)

    xf = x.reshape(TOK, H)
    in_maps = []
    for c in range(N_CORES):
        xs = xf[TOK_PC * c : TOK_PC * (c + 1)]           # [1024, 4096]
        # -> [xc, p, i*1024+t] with h = xc*512 + i*128 + p
        xpk = np.ascontiguousarray(
            xs.T.reshape(N_XCH, XCH, 128, TOK_PC)
            .transpose(0, 2, 1, 3)
            .reshape(N_XCH, 128, XCH * TOK_PC)
            .astype(BF16NP)
        )
        in_maps.append({"xp": xpk, "gwp": gwp, "uwp": uwp, "dwp": dwp})

    res = run_bass_kernel_spmd(nc, in_maps, core_ids=list(range(N_CORES)))

    out = np.empty((TOK, K, H), dtype=np.float32)
    for c in range(N_CORES):
        # [k*32+i, p, t] -> [t, k, i*128+p]
        part = res.results[c]["out"].reshape(K, N_HT, 128, TOK_PC)
        out[TOK_PC * c : TOK_PC * (c + 1)] = (
            part.transpose(3, 0, 1, 2).reshape(TOK_PC, K, H)
        )
    return out.reshape(B, S, K, H)
